# revision 4
# baseline (speedup 1.0000x reference)
"""Single-head causal attention (B=4, T=2048, C=1024, H=128) on 8 trn2 cores.

Sharding: data-parallel over (batch, query-half). core c -> batch c//2,
query group c%2. Query rows are split causally-balanced: group 0 owns rows
[0,512)+[1536,2048), group 1 owns [512,1536). The host permutes x rows so
each core's own 1024 query rows come first; the key order is permuted the
same way, which makes the causal block structure identical on every core
(SPMD single NEFF). The only per-core difference is a 2-float bias that
zeroes key blocks that are fully masked for that core (applied inside exp).

Math (per core, permuted coords): qT/kT/vT = W.T @ xT via PE with xT built
by PE 128x128 transposes; scores^T[s,t] = kT_blk.T @ qT; E = exp(s/32 + bias)
(ACT, reads PSUM); diagonal 128x512 triangle masks multiplied in on GPSIMD;
out^T accumulated as v.T @ E^T and denom row as ones.T @ E^T on PE; denom
replicated across partitions with a K=1 outer-product matmul; normalize,
PE-transpose back to [t,H], DMA out. All matmuls run as float32r.
"""

import sys

if "/opt/trn_rl_repo" not in sys.path:
    sys.path.insert(0, "/opt/trn_rl_repo")

import numpy as np

B, T, C, H = 4, 2048, 1024, 128
P = 128
TJ = 512                 # t-block (free dim) size
NK = C // P              # 8 contraction chunks
NSB = T // P             # 16 key blocks
TOWN = 1024              # own query rows per core
NJ = TOWN // TJ          # 2 query blocks per core
NEG = -1e30
INV_SCALE = 1.0 / 32.0   # C ** -0.5

# key-block sets per query block j' (see module docstring):
#  j'=0: blocks 0-3 diagonal, 8-11 biased (bias col 0), 4-7 & 12-15 skipped
#  j'=1: 0-3 & 8-11 full, 4-7 diagonal, 12-15 biased (bias col 1)
SSET = {
    0: [0, 1, 2, 3, 8, 9, 10, 11],
    1: list(range(16)),
}
DIAG_BASE = {0: 0, 1: 4}          # diag blocks: [base, base+4); mask M[sb-base]
BIAS_GROUP = {0: {8: 0, 9: 0, 10: 0, 11: 0}, 1: {12: 1, 13: 1, 14: 1, 15: 1}}

_CACHE = {}


def _build_nc():
    import concourse.bacc as bacc
    import concourse.mybir as mybir
    import concourse.tile as tile
    from concourse.masks import make_identity

    f32 = mybir.dt.float32
    f32r = mybir.dt.float32r

    def r(ap):
        return ap.bitcast(f32r)

    nc = bacc.Bacc("TRN2", target_bir_lowering=False, debug=False, num_devices=8)

    x = nc.dram_tensor("x", [T, C], f32, kind="ExternalInput").ap()
    wq = nc.dram_tensor("wq", [C, H], f32, kind="ExternalInput").ap()
    wk = nc.dram_tensor("wk", [C, H], f32, kind="ExternalInput").ap()
    wv = nc.dram_tensor("wv", [C, H], f32, kind="ExternalInput").ap()
    sbias = nc.dram_tensor("sbias", [P, 2], f32, kind="ExternalInput").ap()
    out = nc.dram_tensor("out", [TOWN, H], f32, kind="ExternalOutput").ap()

    Exp = mybir.ActivationFunctionType.Exp

    with tile.TileContext(nc) as tc:
        with (
            tc.tile_pool(name="singles", bufs=1) as singles,
            tc.tile_pool(name="xn", bufs=6) as xn_pool,
            tc.tile_pool(name="etile", bufs=3) as e_pool,
            tc.tile_pool(name="stage", bufs=2) as stage,
            tc.tile_pool(name="pp_pairs", bufs=2, space="PSUM") as pp_pairs,
            tc.tile_pool(name="pp_acc", bufs=1, space="PSUM") as pp_acc,
            tc.tile_pool(name="pp_misc", bufs=2, space="PSUM") as pp_misc,
        ):
            # ---- constants / weights ----
            ident = singles.tile([P, P], f32, tag="ident")
            make_identity(nc, ident)
            ones_f = singles.tile([P, 1], f32, tag="ones_f")
            nc.gpsimd.memset(ones_f, 1.0)
            ones_col = singles.tile([P, 1], f32r, tag="ones_col")
            nc.vector.tensor_copy(out=ones_col, in_=ones_f)
            ones_row = singles.tile([1, P], f32, tag="ones_row")
            nc.gpsimd.memset(ones_row, 1.0)
            warm = singles.tile([P, 1], f32, tag="warm")
            nc.scalar.activation(out=warm, in_=ones_f, func=Exp)

            # diagonal masks M[d][r, u] = 1 if u >= r + 128*d else 0
            masks = []
            for d in range(4):
                mf = singles.tile([P, TJ], f32, tag=f"maskf{d}", name=f"maskf{d}")
                nc.gpsimd.memset(mf, 1.0)
                nc.gpsimd.affine_select(
                    out=mf, in_=mf,
                    compare_op=mybir.AluOpType.is_ge,
                    fill=0.0,
                    base=-P * d,
                    pattern=[[1, TJ]],
                    channel_multiplier=-1,
                )
                m = singles.tile([P, TJ], f32r, tag=f"mask{d}", name=f"mask{d}")
                nc.vector.tensor_copy(out=m, in_=mf)
                masks.append(m)

            sbias_sb = singles.tile([P, 2], f32, tag="sbias")
            nc.sync.dma_start(out=sbias_sb, in_=sbias)

            w_sb = {}
            for name, w in (("wq", wq), ("wk", wk), ("wv", wv)):
                t = singles.tile([P, NK, H], f32r, tag=name)
                nc.gpsimd.dma_start(out=t, in_=w.rearrange("(k p) h -> p k h", p=P))
                w_sb[name] = t

            # alternate PSUM->SBUF copies between DVE and ACT
            cp_state = [0]

            def copy_psum(dst, src):
                if cp_state[0] % 2 == 0:
                    nc.vector.tensor_copy(out=dst, in_=src)
                else:
                    nc.scalar.copy(out=dst, in_=src)
                cp_state[0] += 1

            # ---- phase 1: load x, build xT[J] = x^T slabs ----
            xT = [singles.tile([P, NK, TJ], f32r, tag=f"xT{J}", name=f"xT{J}") for J in range(4)]
            for J in range(4):
                xts = []
                for di in range(4):
                    i = 4 * J + di
                    xt = xn_pool.tile([P, C], f32, tag="xn")
                    nc.sync.dma_start(out=xt, in_=x[P * i:P * (i + 1), :])
                    xts.append(xt)
                for k in range(NK):
                    ps = pp_misc.tile([P, TJ], f32, tag="misc")
                    for di in range(4):
                        nc.tensor.transpose(
                            ps[:, P * di:P * (di + 1)],
                            xts[di][:, P * k:P * (k + 1)],
                            ident,
                        )
                    copy_psum(xT[J][:, k, :], ps)

            # ---- phase 2: projections ----
            qT = [singles.tile([P, TJ], f32r, tag=f"qT{j}", name=f"qT{j}") for j in range(NJ)]
            kT = [singles.tile([P, TJ], f32r, tag=f"kT{J}", name=f"kT{J}") for J in range(4)]
            vN = [singles.tile([P, 4, H], f32r, tag=f"vN{J}", name=f"vN{J}") for J in range(4)]

            def project(wname, dst, J):
                ps = pp_misc.tile([P, TJ], f32, tag="misc")
                for k in range(NK):
                    nc.tensor.matmul(
                        ps, w_sb[wname][:, k, :], xT[J][:, k, :],
                        start=(k == 0), stop=(k == NK - 1),
                    )
                copy_psum(dst, ps)

            for J in range(4):
                if J < NJ:
                    project("wq", qT[J], J)
                project("wk", kT[J], J)
                vT = stage.tile([P, TJ], f32, tag="vT")
                project("wv", vT, J)
                ps = pp_misc.tile([P, TJ], f32, tag="misc")
                for di in range(4):
                    nc.tensor.transpose(
                        ps[:, P * di:P * (di + 1)],
                        vT[:, P * di:P * (di + 1)],
                        ident,
                    )
                copy_psum(vN[J], ps.rearrange("p (d h) -> p d h", d=4))

            # ---- phase 3: attention ----
            oT = [stage.tile([P, TJ], f32, tag=f"oT{j}", name=f"oT{j}") for j in range(NJ)]
            denom = singles.tile([1, TOWN], f32, tag="denom")
            for j in range(NJ):
                sset = SSET[j]
                ps_o = pp_acc.tile([P, TJ], f32, tag="o")
                ps_d = pp_acc.tile([1, TJ], f32, tag="d")
                nmm = len(sset)
                mm = 0
                for pi in range(0, nmm, 2):
                    pair = sset[pi:pi + 2]
                    ps2 = pp_pairs.tile([P, 2, TJ], f32, tag="s2")
                    for ri, sb in enumerate(pair):
                        nc.tensor.matmul(
                            ps2[:, ri, :],
                            kT[sb // 4][:, P * (sb % 4):P * (sb % 4 + 1)],
                            qT[j],
                            start=True, stop=True,
                        )
                    bg = BIAS_GROUP[j].get(pair[0])
                    bias = sbias_sb[:, bg:bg + 1] if bg is not None else 0.0
                    e2 = e_pool.tile([P, 2, TJ], f32r, tag="e2")
                    nc.scalar.activation(
                        out=e2, in_=ps2, func=Exp, scale=INV_SCALE, bias=bias,
                    )
                    for ri, sb in enumerate(pair):
                        db = DIAG_BASE[j]
                        if db <= sb < db + 4:
                            nc.gpsimd.tensor_tensor(
                                e2[:, ri, :], e2[:, ri, :], masks[sb - db],
                                op=mybir.AluOpType.mult,
                            )
                        nc.tensor.matmul(
                            ps_o, vN[sb // 4][:, sb % 4, :], e2[:, ri, :],
                            start=(mm == 0), stop=(mm == nmm - 1),
                        )
                        nc.tensor.matmul(
                            ps_d, ones_col, e2[:, ri, :],
                            start=(mm == 0), stop=(mm == nmm - 1),
                        )
                        mm += 1
                nc.vector.tensor_copy(out=oT[j], in_=ps_o)
                nc.vector.tensor_copy(out=denom[0:1, TJ * j:TJ * (j + 1)], in_=ps_d)

            # ---- phase 4: normalize + transpose out ----
            recip = singles.tile([1, TOWN], f32, tag="recip")
            nc.vector.reciprocal(out=recip, in_=denom)
            for j in range(NJ):
                rep = pp_misc.tile([P, TJ], f32, tag="misc")
                nc.tensor.matmul(
                    rep, ones_row, recip[0:1, TJ * j:TJ * (j + 1)],
                    start=True, stop=True,
                )
                otn = stage.tile([P, TJ], f32, tag="otn")
                nc.vector.tensor_mul(out=otn, in0=oT[j], in1=rep)
                ps = pp_misc.tile([P, TJ], f32, tag="misc")
                for di in range(4):
                    nc.tensor.transpose(
                        ps[:, P * di:P * (di + 1)],
                        otn[:, P * di:P * (di + 1)],
                        ident,
                    )
                ob = stage.tile([P, 4, H], f32, tag="ob")
                nc.vector.tensor_copy(out=ob, in_=ps.rearrange("p (d h) -> p d h", d=4))
                nc.sync.dma_start(
                    out=out[TJ * j:TJ * (j + 1), :].rearrange("(d p) h -> p d h", p=P),
                    in_=ob,
                )

    nc.compile()
    return nc


def _get_nc():
    if "nc" not in _CACHE:
        _CACHE["nc"] = _build_nc()
    return _CACHE["nc"]


def kernel(x, Wq, Wk, Wv, mask=None):
    from concourse.bass_utils import run_bass_kernel_spmd

    nc = _get_nc()
    x = np.asarray(x, dtype=np.float32)
    Wq = np.ascontiguousarray(np.asarray(Wq, dtype=np.float32))
    Wk = np.ascontiguousarray(np.asarray(Wk, dtype=np.float32))
    Wv = np.ascontiguousarray(np.asarray(Wv, dtype=np.float32))

    in_maps = []
    for c in range(8):
        b, g = c // 2, c % 2
        xb = x[b]
        if g == 0:
            xp = np.concatenate([xb[0:512], xb[1536:2048], xb[512:1536]], axis=0)
            sb = np.array([NEG, 0.0], dtype=np.float32)
        else:
            xp = np.concatenate([xb[512:1536], xb[0:512], xb[1536:2048]], axis=0)
            sb = np.array([0.0, NEG], dtype=np.float32)
        in_maps.append({
            "x": np.ascontiguousarray(xp),
            "wq": Wq, "wk": Wk, "wv": Wv,
            "sbias": np.ascontiguousarray(np.broadcast_to(sb, (P, 2))),
        })

    res = run_bass_kernel_spmd(nc, in_maps, core_ids=list(range(8)))

    out = np.empty((B, T, H), dtype=np.float32)
    for c, rmap in enumerate(res.results):
        b, g = c // 2, c % 2
        o = rmap["out"]
        if g == 0:
            out[b, 0:512] = o[0:512]
            out[b, 1536:2048] = o[512:1024]
        else:
            out[b, 512:1536] = o
    return out
